# revision 1
# baseline (speedup 1.0000x reference)
"""Block-circulant matvec (FFT linear layer) as dense TensorE matmuls on 8 TRN2 cores.

Math: the reference computes, per output block o,
    y[o, :] = sum_j IFFT(FFT(w[o,j]) * FFT(x[j])).real
which is a sum of circular convolutions:
    y[o, a] = sum_{j, b} w[o, j, b] * x[j, (a - b) mod 128]

Rewritten as matmuls: for each phase b and input-block tile jt (4 tiles of 128),
    YT[a, o] += XR(b,jt)[j', a]^T @ WT(b,jt)[j', o]
where XR(b,jt)[j', a] = x[jt*128+j', (a-b) mod 128] (rotated x tile, stationary)
and   WT(b,jt)[j', o] = w[o, jt*128+j', b]          (moving operand, N=512).

Sharding: the 128 phases b are split 16-per-core across 8 cores; each core
accumulates its 64 (b, jt) groups into one PSUM bank [128a x 512o] and writes a
partial YT. The host sums the 8 partials (no collective needed).

On-chip details: the rotated-x tiles (2 MiB) are built by the otherwise-idle
Vector engine from a 256 KiB doubled-x buffer via an overlapping-window AP
(dest[p, q, jt, a] = src[p, jt, q + a]), so HBM DMA is just the 8.4 MiB bf16
weight shard + 256 KiB of x. The per-core phase offset is folded into a
host-side roll of the x buffer so the SPMD program is core-independent.
Weights stream in eight 8-group chunks on the sync-engine HWDGE FIFO (at most
9 in-flight DMAs — the Tile scheduler has only 8 DMA completion-sem lanes, and
an extra DMA stalls on lane reuse); matmul bursts chase each chunk's
completion semaphore, and ~18 dummy warm-up matmuls lift the PE HAM clock
gate to 2.4 GHz before the first real chunk lands.
"""

import numpy as np
import ml_dtypes

O_BLOCKS = 512
I_BLOCKS = 512
BLOCK = 128
N_CORES = 8
B_PER_CORE = BLOCK // N_CORES          # 16 phases per core
JT_TILES = I_BLOCKS // 128             # 4 contraction tiles
N_GROUPS = B_PER_CORE * JT_TILES       # 64 matmul groups per core
CHUNK_GROUPS = (8, 8, 8, 8, 8, 8, 12, 4)
assert sum(CHUNK_GROUPS) == N_GROUPS
N_WARMUP_MM = 18  # dummy matmuls to lift the PE HAM clock-gate before data lands

_BF16 = ml_dtypes.bfloat16

_MODULE_CACHE = {}


def _build_module():
    import concourse.bass as bass
    import concourse.bacc as bacc
    import concourse.mybir as mybir
    from concourse import tile

    nc = bacc.Bacc(
        "TRN2",
        target_bir_lowering=False,
        debug=False,
        enable_asserts=False,
        enable_partition_id=False,
        num_devices=N_CORES,
    )

    xb2_d = nc.dram_tensor(
        "xb2", [128, JT_TILES, 2 * BLOCK], mybir.dt.bfloat16, kind="ExternalInput"
    )
    wt_d = nc.dram_tensor(
        "wt", [128, N_GROUPS, O_BLOCKS], mybir.dt.bfloat16, kind="ExternalInput"
    )
    yt_d = nc.dram_tensor(
        "yt", [BLOCK, O_BLOCKS], mybir.dt.float32, kind="ExternalOutput"
    )

    with tile.TileContext(nc) as tc:
        with (
            tc.tile_pool(name="xbp", bufs=1) as xbp,
            tc.tile_pool(name="xrp", bufs=1) as xrp,
            tc.tile_pool(name="wtp", bufs=len(CHUNK_GROUPS)) as wtp,
            tc.tile_pool(name="psp", bufs=2, space="PSUM") as psp,
            tc.tile_pool(name="outp", bufs=1) as outp,
            tc.tile_pool(name="scrp", bufs=1) as scrp,
        ):
            # PE warm-up: the HAM clock gate holds the PE at 1.2 GHz until it
            # has been busy ~3.4us. Run dummy matmuls on scratch SBUF while the
            # weight stream is still in flight so real matmuls issue at 2.4 GHz.
            scr = scrp.tile([128, O_BLOCKS], mybir.dt.bfloat16)
            nc.gpsimd.memset(scr[:], 0.0)
            ps_warm = psp.tile([BLOCK, O_BLOCKS], mybir.dt.float32)
            for _ in range(N_WARMUP_MM):
                nc.tensor.matmul(ps_warm[:], scr[:, :BLOCK], scr[:], start=True, stop=True)

            xb2_sb = xbp.tile([128, JT_TILES, 2 * BLOCK], mybir.dt.bfloat16)
            # first in the sync-engine HWDGE FIFO: must fully land before the
            # weight stream floods the SDMA queues (a second ring round-robins
            # at packet granularity and starves this small transfer)
            nc.sync.dma_start(xb2_sb[:], xb2_d[:])

            # Build the 64 rotated-x tiles on the idle DVE:
            #   xr[p, q, jt, a] = xb2[p, jt, q + a]
            # Split so the first chunks' tiles are ready as soon as possible.
            xr_sb = xrp.tile([128, N_GROUPS, BLOCK], mybir.dt.bfloat16)
            xr_ap = xr_sb[:]
            xb2_ap = xb2_sb[:]

            def xr_build(q_lo, q_hi):
                dest = bass.AP(
                    tensor=xr_ap.tensor,
                    offset=xr_ap.offset + q_lo * JT_TILES * BLOCK,
                    ap=[
                        xr_ap.ap[0],                     # partition
                        [JT_TILES * BLOCK, q_hi - q_lo],  # q
                        [BLOCK, JT_TILES],               # jt
                        [1, BLOCK],                      # a
                    ],
                )
                src = bass.AP(
                    tensor=xb2_ap.tensor,
                    offset=xb2_ap.offset + q_lo,
                    ap=[
                        xb2_ap.ap[0],                    # partition
                        [1, q_hi - q_lo],                # q (overlapping windows)
                        [2 * BLOCK, JT_TILES],           # jt
                        [1, BLOCK],                      # a
                    ],
                )
                nc.vector.tensor_copy(dest, src)

            xr_build(0, 4)
            xr_build(4, 16)

            ps = psp.tile([BLOCK, O_BLOCKS], mybir.dt.float32)

            g0 = 0
            for n_g in CHUNK_GROUPS:
                wt_sb = wtp.tile([128, n_g, O_BLOCKS], mybir.dt.bfloat16, tag="wchunk")
                nc.sync.dma_start(wt_sb[:], wt_d[:, g0 : g0 + n_g, :])
                for gi in range(n_g):
                    g = g0 + gi
                    nc.tensor.matmul(
                        ps[:],
                        xr_sb[:, g, :],
                        wt_sb[:, gi, :],
                        start=(g == 0),
                        stop=(g == N_GROUPS - 1),
                    )
                g0 += n_g

            # evacuate PSUM in halves; store the halves on the two independent
            # HWDGE rings (ACT ring is idle by now) so the DMA instruction
            # issue and the completion receipts run in parallel
            out_sb = outp.tile([BLOCK, O_BLOCKS], mybir.dt.float32)
            half = O_BLOCKS // 2
            nc.vector.tensor_copy(out_sb[:, :half], ps[:, :half])
            nc.scalar.dma_start(yt_d[:, :half], out_sb[:, :half])
            nc.vector.tensor_copy(out_sb[:, half:], ps[:, half:])
            nc.sync.dma_start(yt_d[:, half:], out_sb[:, half:])

    nc.compile()
    return nc


def _get_module():
    if "nc" not in _MODULE_CACHE:
        _MODULE_CACHE["nc"] = _build_module()
    return _MODULE_CACHE["nc"]


def _prepare_inputs(x, cir_weights):
    xb = np.asarray(x, dtype=np.float32).reshape(I_BLOCKS, BLOCK)
    W = np.asarray(cir_weights, dtype=np.float32)

    # [b, j, o] bf16, contiguous
    WT = np.ascontiguousarray(W.astype(_BF16).transpose(2, 1, 0))

    xx = xb.astype(_BF16).reshape(JT_TILES, 128, BLOCK)  # [jt, j', c]

    in_maps = []
    for c in range(N_CORES):
        # Group order on core c: g = q*JT_TILES + jt with phase b = 16c+15-q,
        # so the on-chip window walk (src offset q+a) sees ascending q.
        # Host-side roll D_c makes the fixed kernel offset correct per core:
        #   xb2_c[j', jt, cc] = xb[jt*128+j', (cc + D_c) mod 128]
        D_c = (-(B_PER_CORE * c) - (B_PER_CORE - 1)) % BLOCK
        rolled = np.roll(xx, -D_c, axis=2)               # [jt, j', c]
        xb2 = np.concatenate([rolled, rolled], axis=2)   # [jt, j', 256]
        xb2 = np.ascontiguousarray(xb2.transpose(1, 0, 2))  # [j', jt, 256]

        sub = WT[c * B_PER_CORE : (c + 1) * B_PER_CORE]  # [b_idx, j, o], b asc
        sub = sub[::-1]                                  # q = 15 - b_idx
        sub = sub.reshape(N_GROUPS, 128, O_BLOCKS)       # [g=(q,jt), j', o]
        wt = np.ascontiguousarray(sub.transpose(1, 0, 2))  # [j', g, o]

        in_maps.append({"xb2": xb2, "wt": wt})
    return in_maps


def kernel(x, cir_weights):
    from concourse.bass_utils import run_bass_kernel_spmd

    nc = _get_module()
    in_maps = _prepare_inputs(x, cir_weights)
    res = run_bass_kernel_spmd(nc, in_maps, core_ids=list(range(N_CORES)))

    yt = np.zeros((BLOCK, O_BLOCKS), dtype=np.float32)
    for r in res.results:
        yt += r["yt"]
    return np.ascontiguousarray(yt.T).reshape(O_BLOCKS * BLOCK)



# revision 2
# speedup vs baseline: 1.3064x; 1.3064x over previous
"""Block-circulant matvec (FFT linear layer) on 8 TRN2 cores — CRT + fp8 edition.

Math: the reference computes, per output block o,
    y[o, :] = sum_j IFFT(FFT(w[o,j]) * FFT(x[j])).real
which is a sum of length-128 circular convolutions:
    y[o, a] = sum_{j, b} w[o, j, b] * x[j, (a - b) mod 128]

CRT split (z^128-1 = (z^64-1)(z^64+1)): with
    w0 = w[..,:64] + w[..,64:]   (cyclic-64 part,   range [0,2), centered by 1)
    w1 = w[..,:64] - w[..,64:]   (negacyclic-64 part, already centered)
    x0 = (x[..,:64] + x[..,64:])/2,  x1 = (x[..,:64] - x[..,64:])/2
    y[a]    = P0[a] + P1[a] + S/2     (a < 64),   S = sum(x)
    y[a+64] = P0[a] - P1[a] + S/2
where P0 = (w0-1) (cyclic conv) x0 plus-const, P1 = w1 (negacyclic conv) x1.

Each half maps to matmuls like the baseline: per phase q and j-tile jt,
    PS[half][a, o] += XR[j', a]^T @ WT[j', o]
with XR an overlapping-window rotated-x tile (stationary, bf16, 64 cols) and
WT the weight slice (moving, float8e3 = e3m4, N=512). The two halves write
disjoint PSUM partition ranges (0-63 / 64-127), so the PE runs the pair as
concurrent col-group-tiled matmuls: W streams at 2x128 elem/cycle and the
weight bytes halve vs bf16 — both the PE time and the HBM time halve.

Centering w0 by 1 and quantizing the residual keeps the e3m4 step at 2^-6
over most of the mass; the removed constant contributes S/2, added on the
host. Measured rel err vs the fp32 reference: ~9e-3 (gate 2e-2).

Sharding: 64 cyclic + 64 negacyclic phases split 8-per-core; the per-core
phase offset is folded into a host-side (anti)cyclic roll of the x window
buffers so the SPMD program is core-independent. Host sums the 8 partial
PSUM images and applies the butterfly + S/2.
"""

import numpy as np
import ml_dtypes

O_BLOCKS = 512
I_BLOCKS = 512
BLOCK = 128
HALF = 64
N_CORES = 8
Q_PER_CORE = HALF // N_CORES           # 8 phases per core per half
JT_TILES = I_BLOCKS // 128             # 4 contraction tiles
JH = JT_TILES * 2                      # jt*2 + half
N_GROUPS = Q_PER_CORE * JT_TILES * 2   # 64 matmul groups per core
CHUNK_GROUPS = (4, 8, 8, 8, 8, 8, 10, 10)
assert sum(CHUNK_GROUPS) == N_GROUPS
N_WARMUP_MM = 12   # dummy matmuls (N=256) to lift the PE HAM clock-gate
WARM_N = 256

_BF16 = ml_dtypes.bfloat16
_F8E3 = ml_dtypes.float8_e3m4

_MODULE_CACHE = {}


def _build_module():
    import concourse.bass as bass
    import concourse.bacc as bacc
    import concourse.mybir as mybir
    from concourse import tile

    nc = bacc.Bacc(
        "TRN2",
        target_bir_lowering=False,
        debug=False,
        enable_asserts=False,
        enable_partition_id=False,
        num_devices=N_CORES,
    )

    xb2_d = nc.dram_tensor(
        "xb2", [128, JH, BLOCK], mybir.dt.bfloat16, kind="ExternalInput"
    )
    wt_d = nc.dram_tensor(
        "wt", [128, N_GROUPS, O_BLOCKS], mybir.dt.float8e3, kind="ExternalInput"
    )
    yt_d = nc.dram_tensor(
        "yt", [BLOCK, O_BLOCKS], mybir.dt.float32, kind="ExternalOutput"
    )

    with tile.TileContext(nc) as tc:
        with (
            tc.tile_pool(name="xbp", bufs=1) as xbp,
            tc.tile_pool(name="xrp", bufs=1) as xrp,
            tc.tile_pool(name="wtp", bufs=len(CHUNK_GROUPS)) as wtp,
            tc.tile_pool(name="psp", bufs=2, space="PSUM") as psp,
            tc.tile_pool(name="outp", bufs=1) as outp,
            tc.tile_pool(name="scrp", bufs=1) as scrp,
        ):
            # PE warm-up: HAM clock gate holds the PE at 1.2 GHz until it has
            # been busy ~3.4us; dummy matmuls while the weight stream is in
            # flight let the real matmuls issue at 2.4 GHz.
            scr = scrp.tile([128, WARM_N], mybir.dt.bfloat16)
            nc.gpsimd.memset(scr[:], 0.0)
            ps_warm = psp.tile([BLOCK, O_BLOCKS], mybir.dt.float32)
            for _ in range(N_WARMUP_MM):
                nc.tensor.matmul(
                    ps_warm[:, :WARM_N], scr[:, :BLOCK], scr[:], start=True, stop=True
                )

            xb2_sb = xbp.tile([128, JH, BLOCK], mybir.dt.bfloat16)
            # first in the sync-engine HWDGE FIFO: must fully land before the
            # weight stream floods the SDMA queues
            nc.sync.dma_start(xb2_sb[:], xb2_d[:])

            # Build the 64 rotated-x tiles on the idle DVE:
            #   xr[p, (q, jh), a] = xb2[p, jh, q + a]      (overlapping windows)
            xr_sb = xrp.tile([128, N_GROUPS, HALF], mybir.dt.bfloat16)
            xr_ap = xr_sb[:]
            xb2_ap = xb2_sb[:]

            def xr_build(q_lo, q_hi):
                dest = bass.AP(
                    tensor=xr_ap.tensor,
                    offset=xr_ap.offset + q_lo * JH * HALF,
                    ap=[
                        xr_ap.ap[0],                 # partition
                        [JH * HALF, q_hi - q_lo],    # q
                        [HALF, JH],                  # jh
                        [1, HALF],                   # a
                    ],
                )
                src = bass.AP(
                    tensor=xb2_ap.tensor,
                    offset=xb2_ap.offset + q_lo,
                    ap=[
                        xb2_ap.ap[0],                # partition
                        [1, q_hi - q_lo],            # q (overlapping windows)
                        [BLOCK, JH],                 # jh
                        [1, HALF],                   # a
                    ],
                )
                nc.vector.tensor_copy(dest, src)

            xr_build(0, 2)
            xr_build(2, Q_PER_CORE)

            ps = psp.tile([BLOCK, O_BLOCKS], mybir.dt.float32)

            g0 = 0
            for n_g in CHUNK_GROUPS:
                wt_sb = wtp.tile([128, n_g, O_BLOCKS], mybir.dt.float8e3, tag="wchunk")
                nc.sync.dma_start(wt_sb[:], wt_d[:, g0 : g0 + n_g, :])
                for gi in range(n_g):
                    g = g0 + gi
                    half = g & 1
                    ps_half = ps[HALF : 2 * HALF, :] if half else ps[:HALF, :]
                    nc.tensor.matmul(
                        ps_half,
                        xr_sb[:, g, :],
                        wt_sb[:, gi, :],
                        start=(g < 2),
                        stop=(g >= N_GROUPS - 2),
                    )
                g0 += n_g

            # evacuate PSUM in column halves; store on the two independent
            # HWDGE rings so issue and completion receipts run in parallel
            out_sb = outp.tile([BLOCK, O_BLOCKS], mybir.dt.float32)
            half_o = O_BLOCKS // 2
            nc.vector.tensor_copy(out_sb[:, :half_o], ps[:, :half_o])
            nc.scalar.dma_start(yt_d[:, :half_o], out_sb[:, :half_o])
            nc.vector.tensor_copy(out_sb[:, half_o:], ps[:, half_o:])
            nc.sync.dma_start(yt_d[:, half_o:], out_sb[:, half_o:])

    nc.compile()
    return nc


def _get_module():
    if "nc" not in _MODULE_CACHE:
        _MODULE_CACHE["nc"] = _build_module()
    return _MODULE_CACHE["nc"]


def _prepare_inputs(x, cir_weights):
    xb = np.asarray(x, dtype=np.float32).reshape(I_BLOCKS, BLOCK)
    W = np.asarray(cir_weights, dtype=np.float32)

    # CRT halves
    x0 = (xb[:, :HALF] + xb[:, HALF:]) * 0.5          # [j, 64]
    x1 = (xb[:, :HALF] - xb[:, HALF:]) * 0.5
    w0c = (W[..., :HALF] + W[..., HALF:]) - 1.0       # centered cyclic part
    w1 = W[..., :HALF] - W[..., HALF:]
    d0q = w0c.astype(_F8E3)                           # [o, j, b]
    w1q = w1.astype(_F8E3)

    in_maps = []
    for c in range(N_CORES):
        # Group (q, jt, half) on core c handles phase b = 8c + 7 - q, so the
        # on-chip window walk (src offset q+a) sees ascending q. The window
        # buffers bake in the per-core shift t = m - (8c+7):
        #   cyclic:     C[j, m] = x0[j, t mod 64]
        #   negacyclic: N[j, m] = x1[j, t mod 64] * (-1)^floor(t/64)
        t = np.arange(BLOCK) - (N_CORES * c + Q_PER_CORE - 1)
        tm = t % HALF
        sgn = np.where((t // HALF) % 2 == 0, 1.0, -1.0).astype(np.float32)
        C = x0[:, tm]                                  # [j, 128]
        Nn = x1[:, tm] * sgn[None, :]
        C4 = C.reshape(JT_TILES, 128, BLOCK)           # [jt, j', m]
        N4 = Nn.reshape(JT_TILES, 128, BLOCK)
        xb2 = np.stack([C4, N4], axis=1)               # [jt, half, j', m]
        xb2 = np.ascontiguousarray(
            xb2.transpose(2, 0, 1, 3).reshape(128, JH, BLOCK)
        ).astype(_BF16)                                # [j', jh, m]

        qsl = slice(N_CORES * c, N_CORES * c + Q_PER_CORE)
        s0 = d0q[:, :, qsl][..., ::-1]                 # [o, j, q], q -> b=8c+7-q
        s1 = w1q[:, :, qsl][..., ::-1]
        s0r = s0.reshape(O_BLOCKS, JT_TILES, 128, Q_PER_CORE)   # [o, jt, j', q]
        s1r = s1.reshape(O_BLOCKS, JT_TILES, 128, Q_PER_CORE)
        st = np.stack([s0r, s1r], axis=0)              # [half, o, jt, j', q]
        wt = np.ascontiguousarray(
            st.transpose(3, 4, 2, 0, 1).reshape(128, N_GROUPS, O_BLOCKS)
        )                                              # [j', (q, jt, half), o]

        in_maps.append({"xb2": xb2, "wt": wt})
    return in_maps


def kernel(x, cir_weights):
    from concourse.bass_utils import run_bass_kernel_spmd

    nc = _get_module()
    in_maps = _prepare_inputs(x, cir_weights)
    res = run_bass_kernel_spmd(nc, in_maps, core_ids=list(range(N_CORES)))

    yt = np.zeros((BLOCK, O_BLOCKS), dtype=np.float64)
    for r in res.results:
        yt += r["yt"]
    s_half = 0.5 * float(np.asarray(x, dtype=np.float64).sum())
    p0 = yt[:HALF]                                     # [64, 512] cyclic
    p1 = yt[HALF:]                                     # [64, 512] negacyclic
    y_low = p0 + p1 + s_half                           # a in [0, 64)
    y_high = p0 - p1 + s_half                          # a in [64, 128)
    ya = np.concatenate([y_low, y_high], axis=0)       # [a 128, o 512]
    return np.ascontiguousarray(ya.T).astype(np.float32).reshape(O_BLOCKS * BLOCK)


# revision 4
# speedup vs baseline: 1.3538x; 1.0363x over previous
"""Block-circulant matvec (FFT linear layer) on 8 TRN2 cores — CRT + fp8 edition.

Math: the reference computes, per output block o,
    y[o, :] = sum_j IFFT(FFT(w[o,j]) * FFT(x[j])).real
which is a sum of length-128 circular convolutions:
    y[o, a] = sum_{j, b} w[o, j, b] * x[j, (a - b) mod 128]

CRT split (z^128-1 = (z^64-1)(z^64+1)): with
    w0 = w[..,:64] + w[..,64:]   (cyclic-64 part,   range [0,2), centered by 1)
    w1 = w[..,:64] - w[..,64:]   (negacyclic-64 part, already centered)
    x0 = (x[..,:64] + x[..,64:])/2,  x1 = (x[..,:64] - x[..,64:])/2
    y[a]    = P0[a] + P1[a] + S/2     (a < 64),   S = sum(x)
    y[a+64] = P0[a] - P1[a] + S/2
where P0 = (w0-1) (cyclic conv) x0 plus-const, P1 = w1 (negacyclic conv) x1.

Each half maps to matmuls like the baseline: per phase q and j-tile jt,
    PS[half][a, o] += XR[j', a]^T @ WT[j', o]
with XR an overlapping-window rotated-x tile (stationary, bf16, 64 cols) and
WT the weight slice (moving, float8e3 = e3m4, N=512). The two halves write
disjoint PSUM partition ranges (0-63 / 64-127), so the PE runs the pair as
concurrent col-group-tiled matmuls: W streams at 2x128 elem/cycle and the
weight bytes halve vs bf16 — both the PE time and the HBM time halve.

Centering w0 by 1 and quantizing the residual keeps the e3m4 step at 2^-6
over most of the mass; the removed constant contributes S/2, added on the
host. Measured rel err vs the fp32 reference: ~9e-3 (gate 2e-2).

Sharding: 64 cyclic + 64 negacyclic phases split 8-per-core; the per-core
phase offset is folded into a host-side (anti)cyclic roll of the x window
buffers so the SPMD program is core-independent. Host sums the 8 partial
PSUM images and applies the butterfly + S/2.
"""

import numpy as np
import ml_dtypes

O_BLOCKS = 512
I_BLOCKS = 512
BLOCK = 128
HALF = 64
N_CORES = 8
Q_PER_CORE = HALF // N_CORES           # 8 phases per core per half
JT_TILES = I_BLOCKS // 128             # 4 contraction tiles
JH = JT_TILES * 2                      # jt*2 + half
N_GROUPS = Q_PER_CORE * JT_TILES * 2   # 64 matmul groups per core
CHUNK_GROUPS = (4, 4, 8, 16, 16, 16)
assert sum(CHUNK_GROUPS) == N_GROUPS
N_WARMUP_MM = 24   # dummy matmuls (N=256) to lift the PE HAM clock-gate
WARM_N = 256

_BF16 = ml_dtypes.bfloat16
_F8E3 = ml_dtypes.float8_e3m4

_MODULE_CACHE = {}


def _build_module():
    import concourse.bass as bass
    import concourse.bacc as bacc
    import concourse.mybir as mybir
    from concourse import tile

    nc = bacc.Bacc(
        "TRN2",
        target_bir_lowering=False,
        debug=False,
        enable_asserts=False,
        enable_partition_id=False,
        num_devices=N_CORES,
    )

    xb2_d = nc.dram_tensor(
        "xb2", [128, JH, BLOCK], mybir.dt.bfloat16, kind="ExternalInput"
    )
    wt_d = nc.dram_tensor(
        "wt", [128, N_GROUPS, O_BLOCKS], mybir.dt.float8e3, kind="ExternalInput"
    )
    yt_d = nc.dram_tensor(
        "yt", [BLOCK, O_BLOCKS], mybir.dt.float32, kind="ExternalOutput"
    )

    with tile.TileContext(nc) as tc:
        with (
            tc.tile_pool(name="xbp", bufs=1) as xbp,
            tc.tile_pool(name="xrp", bufs=1) as xrp,
            tc.tile_pool(name="wtp", bufs=len(CHUNK_GROUPS)) as wtp,
            tc.tile_pool(name="psp", bufs=2, space="PSUM") as psp,
            tc.tile_pool(name="outp", bufs=1) as outp,
            tc.tile_pool(name="scrp", bufs=1) as scrp,
        ):
            # PE warm-up: HAM clock gate holds the PE at 1.2 GHz until it has
            # been busy ~3.4us; dummy matmuls while the weight stream is in
            # flight let the real matmuls issue at 2.4 GHz.
            scr = scrp.tile([128, WARM_N], mybir.dt.bfloat16)
            nc.gpsimd.memset(scr[:], 0.0)
            ps_warm = psp.tile([BLOCK, O_BLOCKS], mybir.dt.float32)
            for _ in range(N_WARMUP_MM):
                nc.tensor.matmul(
                    ps_warm[:, :WARM_N], scr[:, :BLOCK], scr[:], start=True, stop=True
                )

            xb2_sb = xbp.tile([128, JH, BLOCK], mybir.dt.bfloat16)
            # on the ACT HWDGE ring (idle until the output store) so the
            # weight chunks start issuing on the sync ring immediately
            nc.scalar.dma_start(xb2_sb[:], xb2_d[:])

            # Build the 64 rotated-x tiles on the idle DVE:
            #   xr[p, (q, jh), a] = xb2[p, jh, q + a]      (overlapping windows)
            xr_sb = xrp.tile([128, N_GROUPS, HALF], mybir.dt.bfloat16)
            xr_ap = xr_sb[:]
            xb2_ap = xb2_sb[:]

            def xr_build(q_lo, q_hi):
                dest = bass.AP(
                    tensor=xr_ap.tensor,
                    offset=xr_ap.offset + q_lo * JH * HALF,
                    ap=[
                        xr_ap.ap[0],                 # partition
                        [JH * HALF, q_hi - q_lo],    # q
                        [HALF, JH],                  # jh
                        [1, HALF],                   # a
                    ],
                )
                src = bass.AP(
                    tensor=xb2_ap.tensor,
                    offset=xb2_ap.offset + q_lo,
                    ap=[
                        xb2_ap.ap[0],                # partition
                        [1, q_hi - q_lo],            # q (overlapping windows)
                        [BLOCK, JH],                 # jh
                        [1, HALF],                   # a
                    ],
                )
                nc.vector.tensor_copy(dest, src)

            xr_build(0, 2)
            xr_build(2, Q_PER_CORE)

            ps = psp.tile([BLOCK, O_BLOCKS], mybir.dt.float32)

            g0 = 0
            for n_g in CHUNK_GROUPS:
                wt_sb = wtp.tile([128, n_g, O_BLOCKS], mybir.dt.float8e3, tag="wchunk")
                nc.sync.dma_start(wt_sb[:], wt_d[:, g0 : g0 + n_g, :])
                for gi in range(n_g):
                    g = g0 + gi
                    half = g & 1
                    ps_half = ps[HALF : 2 * HALF, :] if half else ps[:HALF, :]
                    nc.tensor.matmul(
                        ps_half,
                        xr_sb[:, g, :],
                        wt_sb[:, gi, :],
                        start=(g < 2),
                        stop=(g >= N_GROUPS - 2),
                    )
                g0 += n_g

            # evacuate PSUM in column halves; store on the two independent
            # HWDGE rings so issue and completion receipts run in parallel
            out_sb = outp.tile([BLOCK, O_BLOCKS], mybir.dt.float32)
            half_o = O_BLOCKS // 2
            nc.vector.tensor_copy(out_sb[:, :half_o], ps[:, :half_o])
            nc.scalar.dma_start(yt_d[:, :half_o], out_sb[:, :half_o])
            nc.vector.tensor_copy(out_sb[:, half_o:], ps[:, half_o:])
            nc.sync.dma_start(yt_d[:, half_o:], out_sb[:, half_o:])

    nc.compile()
    return nc


def _get_module():
    if "nc" not in _MODULE_CACHE:
        _MODULE_CACHE["nc"] = _build_module()
    return _MODULE_CACHE["nc"]


def _prepare_inputs(x, cir_weights):
    xb = np.asarray(x, dtype=np.float32).reshape(I_BLOCKS, BLOCK)
    W = np.asarray(cir_weights, dtype=np.float32)

    # CRT halves
    x0 = (xb[:, :HALF] + xb[:, HALF:]) * 0.5          # [j, 64]
    x1 = (xb[:, :HALF] - xb[:, HALF:]) * 0.5
    w0c = (W[..., :HALF] + W[..., HALF:]) - 1.0       # centered cyclic part
    w1 = W[..., :HALF] - W[..., HALF:]
    d0q = w0c.astype(_F8E3)                           # [o, j, b]
    w1q = w1.astype(_F8E3)

    in_maps = []
    for c in range(N_CORES):
        # Group (q, jt, half) on core c handles phase b = 8c + 7 - q, so the
        # on-chip window walk (src offset q+a) sees ascending q. The window
        # buffers bake in the per-core shift t = m - (8c+7):
        #   cyclic:     C[j, m] = x0[j, t mod 64]
        #   negacyclic: N[j, m] = x1[j, t mod 64] * (-1)^floor(t/64)
        t = np.arange(BLOCK) - (N_CORES * c + Q_PER_CORE - 1)
        tm = t % HALF
        sgn = np.where((t // HALF) % 2 == 0, 1.0, -1.0).astype(np.float32)
        C = x0[:, tm]                                  # [j, 128]
        Nn = x1[:, tm] * sgn[None, :]
        C4 = C.reshape(JT_TILES, 128, BLOCK)           # [jt, j', m]
        N4 = Nn.reshape(JT_TILES, 128, BLOCK)
        xb2 = np.stack([C4, N4], axis=1)               # [jt, half, j', m]
        xb2 = np.ascontiguousarray(
            xb2.transpose(2, 0, 1, 3).reshape(128, JH, BLOCK)
        ).astype(_BF16)                                # [j', jh, m]

        qsl = slice(N_CORES * c, N_CORES * c + Q_PER_CORE)
        s0 = d0q[:, :, qsl][..., ::-1]                 # [o, j, q], q -> b=8c+7-q
        s1 = w1q[:, :, qsl][..., ::-1]
        s0r = s0.reshape(O_BLOCKS, JT_TILES, 128, Q_PER_CORE)   # [o, jt, j', q]
        s1r = s1.reshape(O_BLOCKS, JT_TILES, 128, Q_PER_CORE)
        st = np.stack([s0r, s1r], axis=0)              # [half, o, jt, j', q]
        wt = np.ascontiguousarray(
            st.transpose(3, 4, 2, 0, 1).reshape(128, N_GROUPS, O_BLOCKS)
        )                                              # [j', (q, jt, half), o]

        in_maps.append({"xb2": xb2, "wt": wt})
    return in_maps


def kernel(x, cir_weights):
    from concourse.bass_utils import run_bass_kernel_spmd

    nc = _get_module()
    in_maps = _prepare_inputs(x, cir_weights)
    res = run_bass_kernel_spmd(nc, in_maps, core_ids=list(range(N_CORES)))

    yt = np.zeros((BLOCK, O_BLOCKS), dtype=np.float64)
    for r in res.results:
        yt += r["yt"]
    s_half = 0.5 * float(np.asarray(x, dtype=np.float64).sum())
    p0 = yt[:HALF]                                     # [64, 512] cyclic
    p1 = yt[HALF:]                                     # [64, 512] negacyclic
    y_low = p0 + p1 + s_half                           # a in [0, 64)
    y_high = p0 - p1 + s_half                          # a in [64, 128)
    ya = np.concatenate([y_low, y_high], axis=0)       # [a 128, o 512]
    return np.ascontiguousarray(ya.T).astype(np.float32).reshape(O_BLOCKS * BLOCK)
